# revision 2
# baseline (speedup 1.0000x reference)
"""DA-RNN (dual-stage attention RNN) on 8 Trainium2 NeuronCores.

Sharding: data-parallel over batch (B=256 -> 8 shards of 32); all weights
replicated; the sequential encoder/decoder scans stay local per shard.
Each shard's full forward pass is compiled once and dispatched to its core
via jax pmap over the 8 axon-tunneled NeuronCores.
"""
import numpy as np
import jax
import jax.numpy as jnp

# Persistent compile cache: the scan-heavy graph is slow to compile on
# neuronx-cc, so cache the executable across processes.
try:
    jax.config.update("jax_compilation_cache_dir", "/root/.jax_kernel_cache")
    jax.config.update("jax_persistent_cache_min_entry_size_bytes", -1)
    jax.config.update("jax_persistent_cache_min_compile_time_secs", 0.0)
except Exception:
    pass

# Problem dims (hardcoded per spec nn_DARNN_45749991637072)
T, P, N, M, PD = 64, 16, 96, 512, 512
B = 256
TT = T + P
NCORES = 8
BS = B // NCORES  # 32 rows per core

WEIGHT_NAMES = [
    "W_ih_e", "W_hh_e", "b_ih_e", "b_hh_e", "v_e", "W_e", "U_e",
    "W_ih_d", "W_hh_d", "b_ih_d", "b_hh_d", "v_d", "W_d", "U_d",
    "w_tilde_w", "w_tilde_b", "lin1_w", "lin1_b", "lin2_w", "lin2_b",
]


def _lstm_step(x, h, c, W_ih, W_hh, b_ih, b_hh):
    g = x @ W_ih.T + b_ih + h @ W_hh.T + b_hh
    i, f, gg, o = jnp.split(g, 4, axis=-1)
    c = jax.nn.sigmoid(f) * c + jax.nn.sigmoid(i) * jnp.tanh(gg)
    h = jax.nn.sigmoid(o) * jnp.tanh(c)
    return h, c


def _forward(enc_data, dec_data, W_ih_e, W_hh_e, b_ih_e, b_hh_e, v_e, W_e, U_e,
             W_ih_d, W_hh_d, b_ih_d, b_hh_d, v_d, W_d, U_d,
             w_tilde_w, w_tilde_b, lin1_w, lin1_b, lin2_w, lin2_b):
    Bx = enc_data.shape[0]
    UE = jnp.einsum('btn,st->bns', enc_data, U_e)

    def enc_step(carry, x_t):
        h, c = carry
        q = jnp.concatenate([h, c], axis=-1) @ W_e.T
        e = jnp.tanh(q[:, None, :] + UE) @ v_e
        alpha = jax.nn.softmax(e, axis=-1)
        h, c = _lstm_step(x_t * alpha, h, c, W_ih_e, W_hh_e, b_ih_e, b_hh_e)
        return (h, c), h

    h0 = jnp.zeros((Bx, M), dtype=enc_data.dtype)
    _, H = jax.lax.scan(enc_step, (h0, h0), jnp.swapaxes(enc_data, 0, 1))
    H = jnp.swapaxes(H, 0, 1)

    UH = H @ U_d.T

    def attn_ctx(d, s):
        q = jnp.concatenate([d, s], axis=-1) @ W_d.T
        e = jnp.tanh(q[:, None, :] + UH) @ v_d
        beta = jax.nn.softmax(e, axis=1)
        return jnp.einsum('bt,btm->bm', beta, H)

    def dec_step(carry, y_t):
        d, s = carry
        ctx = attn_ctx(d, s)
        y_til = jnp.concatenate([y_t, ctx], axis=-1) @ w_tilde_w.T + w_tilde_b
        d, s = _lstm_step(y_til, d, s, W_ih_d, W_hh_d, b_ih_d, b_hh_d)
        return (d, s), None

    d0 = jnp.zeros((Bx, PD), dtype=enc_data.dtype)
    (d, s), _ = jax.lax.scan(dec_step, (d0, d0), jnp.swapaxes(dec_data, 0, 1))
    ctx = attn_ctx(d, s)
    out = jnp.concatenate([d, ctx], axis=-1)
    out = jax.nn.relu(out @ lin1_w.T + lin1_b) @ lin2_w.T + lin2_b
    return out


_pmapped = None


def _get_pmapped():
    global _pmapped
    if _pmapped is None:
        in_axes = (0, 0) + (None,) * len(WEIGHT_NAMES)
        _pmapped = jax.pmap(_forward, in_axes=in_axes,
                            devices=jax.devices()[:NCORES])
    return _pmapped


def kernel(**inputs):
    enc = np.asarray(inputs["enc_data"], dtype=np.float32)
    dec = np.asarray(inputs["dec_data"], dtype=np.float32)
    Bx = enc.shape[0]
    bs = Bx // NCORES
    enc_sh = enc.reshape(NCORES, bs, enc.shape[1], enc.shape[2])
    dec_sh = dec.reshape(NCORES, bs, dec.shape[1], dec.shape[2])
    ws = [np.asarray(inputs[k], dtype=np.float32) for k in WEIGHT_NAMES]
    fn = _get_pmapped()
    out = fn(enc_sh, dec_sh, *ws)          # [8, bs, P]
    out = np.asarray(jax.device_get(out)).reshape(Bx, -1).astype(np.float32)
    return out


if __name__ == "__main__":
    # smoke test with random inputs of the right shapes
    rng = np.random.default_rng(0)
    dummy = dict(
        enc_data=rng.standard_normal((B, TT, N), dtype=np.float32),
        dec_data=rng.standard_normal((B, T - 1, 1), dtype=np.float32),
    )
    shapes = dict(
        W_ih_e=(4 * M, N), W_hh_e=(4 * M, M), b_ih_e=(4 * M,), b_hh_e=(4 * M,),
        v_e=(TT,), W_e=(TT, 2 * M), U_e=(TT, TT),
        W_ih_d=(4 * PD, 1), W_hh_d=(4 * PD, PD), b_ih_d=(4 * PD,), b_hh_d=(4 * PD,),
        v_d=(M,), W_d=(M, 2 * PD), U_d=(M, M),
        w_tilde_w=(1, M + 1), w_tilde_b=(1,),
        lin1_w=(PD, M + PD), lin1_b=(PD,), lin2_w=(P, PD), lin2_b=(P,),
    )
    for k, shp in shapes.items():
        dummy[k] = (rng.standard_normal(shp) * 0.05).astype(np.float32)
    out = kernel(**dummy)
    print("out", out.shape, out.dtype, float(np.abs(out).max()))
